# revision 31
# baseline (speedup 1.0000x reference)
"""MultiHeadedAttention Trainium2 kernel (8-core SPMD, data-parallel).

Sharding: 8 cores = (batch b in 0..3) x (query half in 0..1). Each core
computes out[b, half*1024:(half+1)*1024, :] independently - no collectives.

Per-core dataflow (all "T" = transposed layouts, contraction dim on partitions):
  - inputs cast fp32->bf16 during DMA (SWDGE), staged to DRAM, transposed
    back into SBUF via xbar DMA transpose in 512-col slices
  - projections (bf16 matmuls, fp32 psum): qT/kT [d_head, S] per head-pair,
    v natural [Sk, d] with a trailing ones column per head (Z trick)
  - scores^T [Sk-tile, Sq-slice] = kT.T @ qT per head (K=64, two heads
    row-tiled concurrently); exp on ScalarE (scale 1/8 folded into Wq/bq,
    no max-subtraction needed: |s/8| <~ 2 by construction); mask applied
    as bf16 multiply with maskT (staged transpose of int mask)
  - PV: psum rows 0..63 = sum_j v^T p, row 64 = Z (ones col); finalize:
    PE-broadcast Z, reciprocal_approx_fast, multiply, SBUF->SBUF DMA hop
    into head-pair layout xattnT [dm, Sq]
  - out = xattnT.T @ WoT + R where R = bo + bv@WoT (PE-broadcast), fp32
"""
import numpy as np
import ml_dtypes

import concourse.bass as bass
import concourse.mybir as mybir
import concourse.tile as tile
from concourse import bacc
from concourse.bass_utils import run_bass_kernel_spmd

F32 = mybir.dt.float32
BF16 = mybir.dt.bfloat16
I32 = mybir.dt.int32
AF = mybir.ActivationFunctionType
ALU = mybir.AluOpType

N_CORES = 8
DK = 64


def slices(total, chunk):
    return [(s, min(chunk, total - s)) for s in range(0, total, chunk)]


class Cfg:
    def __init__(self, SQ=1024, SK=2048, DM=1024, H=16, max_stage=5):
        assert DM % 128 == 0 and SK % 128 == 0 and SQ % 128 == 0 and H % 2 == 0
        self.SQ, self.SK, self.DM, self.H = SQ, SK, DM, H
        self.KT = DM // 128          # dm contraction chunks
        self.HP = H // 2             # head pairs
        self.NJ = SK // 128          # Sk tiles
        self.SQS = min(1024, SQ)     # attention Sq slice width (2 psum banks)
        self.max_stage = max_stage   # debug: truncate kernel after stage N
        assert SQ % self.SQS == 0
        assert H * DK == DM


def emit_kernel(tc, cfg, io):
    nc = tc.nc
    C = cfg
    xq, xk, xv, msk = io["xq"], io["xk"], io["xv"], io["mask"]
    w_dram = {"q": io["wqt"], "k": io["wkt"], "v": io["wvt"], "o": io["wot"]}
    bql, bkl, bvl, bo_row = io["bql"], io["bkl"], io["bvl"], io["bo_row"]
    out = io["out"]

    pools = {}

    def open_pool(name, bufs=1, space="SBUF"):
        pools[name] = tc.alloc_tile_pool(name=name, bufs=bufs, space=space)
        return pools[name]

    persist = open_pool("persist", 1)
    dram = open_pool("dram", 1, space="DRAM")
    wo_pool = open_pool("wo", 1)
    # 8 banks: "s" 2 slots x 2 banks + "pv" 2 slots x 2 banks; zb/R/outproj
    # psums share the "s" slots (attention and epilogue barely overlap them)
    ps_s = open_pool("ps_s", 2, space="PSUM")
    ps_pv = open_pool("ps_pv", 2, space="PSUM")
    ps_zb = ps_s
    ps_out = ps_s
    wx_pool = open_pool("wx", 2)
    staging = open_pool("staging", 1)
    xts_pool = open_pool("xts", 2)

    # ---------------- persistent tiles (~102 KB/partition) ----------------
    qT_sb = persist.tile([128, C.HP * C.SQ], BF16, name="qT_sb")
    kT_sb = persist.tile([128, C.HP * C.SK], BF16, name="kT_sb")
    v_sb = persist.tile([128, C.NJ * C.H * 65], BF16, name="v_sb")
    xattnT_sb = persist.tile([128, C.HP * C.SQ], BF16, name="xattnT_sb")
    R_sb = persist.tile([128, C.DM], F32, name="R_sb")
    bql_sb = persist.tile([128, C.HP], F32, name="bql_sb")
    bkl_sb = persist.tile([128, C.HP], F32, name="bkl_sb")
    bvl_sb = persist.tile([128, C.KT], BF16, name="bvl_sb")
    bo_sb = persist.tile([1, C.DM], F32, name="bo_sb")
    onesf_sb = persist.tile([65, 128], F32, name="onesf_sb")
    Rrow_sb = persist.tile([1, C.DM], F32, name="Rrow_sb")

    nc.sync.dma_start(bql_sb[:], bql[:])
    nc.sync.dma_start(bkl_sb[:], bkl[:])
    nc.sync.dma_start(bvl_sb[:], bvl[:])
    nc.sync.dma_start(bo_sb[:], bo_row[:])
    nc.vector.memset(onesf_sb[:], 1.0)

    wo_sb = wo_pool.tile([128, C.KT * C.DM], BF16, name="wo_sb")
    for kt in range(C.KT):
        nc.scalar.dma_start(wo_sb[:, kt * C.DM:(kt + 1) * C.DM],
                            w_dram["o"][kt * 128:(kt + 1) * 128, :])

    PS_F = max(C.SQS, 512)  # tag-"s" psum slot free-size (2 banks at 1024)

    # stage helpers: cast+stage a tensor (gpsimd cast-load, scalar store),
    # then xbar-transpose slices back on the SP queue (SP handles ONLY xbar).
    stg = {}

    def stage1_x(name, x_in, S):
        stg[name] = dram.tile([S, C.DM], BF16, name=f"stg_{name}",
                              uniquify=True)
        for st in range(S // 128):
            t = staging.tile([128, C.DM], BF16, name="xcast", tag="xcast",
                             bufs=3, padded_shape=[128, max(C.DM, 1024)])
            nc.gpsimd.dma_start(t[:], x_in[st * 128:(st + 1) * 128, :])
            nc.sync.dma_start(stg[name][st * 128:(st + 1) * 128, :], t[:])

    def load_xT_slice(name, ns, nw):
        """xT slice [128, KT*nw]: block kt = x[ns:ns+nw, kt*128:(kt+1)*128]^T."""
        xt = xts_pool.tile([128, C.KT * nw], BF16, name=f"xT_{name}", tag="xts",
                           padded_shape=[128, C.KT * 512])
        for kt in range(C.KT):
            nc.sync.dma_start(
                xt[:, kt * nw:(kt + 1) * nw],
                stg[name][ns:ns + nw, kt * 128:(kt + 1) * 128],
                transpose=True,
            )
        return xt

    def finish():
        for pl in reversed(list(pools.values())):
            pl.release()

    # ---------------- k: stage then project (kT per head-pair) ----------------
    stage1_x("k", xk, C.SK)
    wk_sb = wx_pool.tile([128, C.KT * C.DM], BF16, name="w_k", tag="w")
    for kt in range(C.KT):
        nc.scalar.dma_start(wk_sb[:, kt * C.DM:(kt + 1) * C.DM],
                            w_dram["k"][kt * 128:(kt + 1) * 128, :])
    for (ns, nw) in slices(C.SK, 512):
        xt = load_xT_slice("k", ns, nw)
        for hp in range(C.HP):
            ps = ps_s.tile([128, nw], F32, name="ps_proj", tag="s",
                           padded_shape=[128, PS_F])
            for kt in range(C.KT):
                nc.tensor.matmul(
                    ps[:],
                    wk_sb[:, kt * C.DM + hp * 128: kt * C.DM + (hp + 1) * 128],
                    xt[:, kt * nw:(kt + 1) * nw],
                    start=(kt == 0), stop=(kt == C.KT - 1),
                )
            nc.scalar.activation(
                kT_sb[:, hp * C.SK + ns: hp * C.SK + ns + nw], ps[:],
                AF.Identity, bias=bkl_sb[:, hp:hp + 1],
            )

    # ---------------- q: stage then project ----------------
    stage1_x("q", xq, C.SQ)
    wq_sb = wx_pool.tile([128, C.KT * C.DM], BF16, name="w_q", tag="w")
    for kt in range(C.KT):
        nc.scalar.dma_start(wq_sb[:, kt * C.DM:(kt + 1) * C.DM],
                          w_dram["q"][kt * 128:(kt + 1) * 128, :])
    for (ns, nw) in slices(C.SQ, 512):
        xt = load_xT_slice("q", ns, nw)
        for hp in range(C.HP):
            ps = ps_s.tile([128, nw], F32, name="ps_qp", tag="s",
                           padded_shape=[128, PS_F])
            for kt in range(C.KT):
                nc.tensor.matmul(
                    ps[:],
                    wq_sb[:, kt * C.DM + hp * 128: kt * C.DM + (hp + 1) * 128],
                    xt[:, kt * nw:(kt + 1) * nw],
                    start=(kt == 0), stop=(kt == C.KT - 1),
                )
            nc.scalar.activation(
                qT_sb[:, hp * C.SQ + ns: hp * C.SQ + ns + nw], ps[:],
                AF.Identity, bias=bql_sb[:, hp:hp + 1],
            )

    # ---------------- v: stage then project ----------------
    # v natural [Sk, d] + ones col: v_sb block j: [128, H*65], head h at
    # cols [65h, 65h+65): cols 65h..65h+63 = v dims, col 65h+64 = ones
    # (so the PV matmul's psum row 64 = Z; v-bias folded into R)
    stage1_x("v", xv, C.SK)
    wv_sb = wx_pool.tile([128, C.KT * C.DM], BF16, name="w_v", tag="w")
    for kt in range(C.KT):
        nc.scalar.dma_start(wv_sb[:, kt * C.DM:(kt + 1) * C.DM],
                            w_dram["v"][kt * 128:(kt + 1) * 128, :])
    v_view = v_sb.rearrange("p (j h c) -> p j h c", j=C.NJ, c=65)
    for (ns, nw) in slices(C.SK, 512):
        xt = load_xT_slice("v", ns, nw)
        for j in range(ns // 128, (ns + nw) // 128):
            jo = j * 128 - ns
            for (ds_, dw) in slices(C.DM, 512):
                hs, hw = ds_ // DK, dw // DK
                ps = ps_s.tile([128, dw], F32, name="ps_v", tag="s",
                               padded_shape=[128, PS_F])
                for kt in range(C.KT):
                    nc.tensor.matmul(
                        ps[:],
                        xt[:, kt * nw + jo: kt * nw + jo + 128],
                        wv_sb[:, kt * C.DM + ds_: kt * C.DM + ds_ + dw],
                        start=(kt == 0), stop=(kt == C.KT - 1),
                    )
                nc.vector.tensor_copy(
                    v_view[:, j, hs:hs + hw, 0:64],
                    ps.rearrange("p (h c) -> p h c", c=DK),
                )
    nc.vector.memset(v_view[:, :, :, 64:65], 1.0)

    # ---------------- mask staging ----------------
    mstg = dram.tile([C.SQ, C.SK], BF16, name="mstg")
    mchunk = min(1024, C.SK)
    for st in range(C.SQ // 128):
        for (cs, cw) in slices(C.SK, mchunk):
            ti = staging.tile([128, cw], I32, name="mint", tag="mint", bufs=2,
                              padded_shape=[128, mchunk])
            nc.sync.dma_start(ti[:], msk[st * 128:(st + 1) * 128, cs:cs + cw])
            tb = staging.tile([128, cw], BF16, name="mbf", tag="mbf", bufs=2,
                              padded_shape=[128, mchunk])
            nc.vector.tensor_copy(tb[:], ti[:])
            nc.sync.dma_start(mstg[st * 128:(st + 1) * 128, cs:cs + cw], tb[:])

    if C.max_stage <= 2:
        finish()
        return

    # ---------------- stage 3: attention ----------------
    xts_pool.release()
    del pools["xts"]
    staging.release()
    del pools["staging"]
    wx_pool.release()
    del pools["wx"]
    attn = open_pool("attn", 1)
    for (sq, sw) in slices(C.SQ, C.SQS):
        # maskT slice [Sk, sw] via xbar, block j at cols [j*sw, (j+1)*sw)
        maskT = attn.tile([128, C.NJ * sw], BF16, name="maskT", tag="maskT",
                          bufs=1, padded_shape=[128, C.NJ * C.SQS])
        for j in range(C.NJ):
            nc.sync.dma_start(
                maskT[:, j * sw:(j + 1) * sw],
                mstg[sq:sq + sw, j * 128:(j + 1) * 128],
                transpose=True,
            )
        for hp in range(C.HP):
            pv = [
                ps_pv.tile([65, sw], F32, name=f"ps_pv{i}", tag="pv",
                           padded_shape=[65, PS_F])
                for i in range(2)
            ]
            # software pipeline: scores/exp/mask run PIPE iterations ahead of
            # the PV matmuls so the in-order PE stream never stalls on the
            # ACT(exp) -> DVE(mask) chain of its own iteration.
            PIPE = 1
            pm_hist = []

            def emit_pv(jj, pms):
                for i in range(2):
                    for (qs, qw) in slices(sw, 512):
                        nc.tensor.matmul(
                            pv[i][:, qs:qs + qw], v_view[:, jj, 2 * hp + i, :],
                            pms[i][:, qs:qs + qw],
                            start=(jj == 0), stop=(jj == C.NJ - 1),
                        )

            for j in range(C.NJ):
                pms = []
                for i in range(2):
                    ss = ps_s.tile([128, sw], F32, name="ps_sc", tag="s",
                                   padded_shape=[128, PS_F])
                    for (qs, qw) in slices(sw, 512):
                        nc.tensor.matmul(
                            ss[:, qs:qs + qw],
                            kT_sb[i * 64:(i + 1) * 64,
                                  hp * C.SK + j * 128: hp * C.SK + (j + 1) * 128],
                            qT_sb[i * 64:(i + 1) * 64,
                                  hp * C.SQ + sq + qs: hp * C.SQ + sq + qs + qw],
                            start=True, stop=True,
                        )
                    pe = attn.tile([128, sw], BF16, name="p_exp", tag="pexp",
                                   bufs=2, padded_shape=[128, C.SQS])
                    nc.scalar.activation(pe[:], ss[:], AF.Exp)
                    pm = attn.tile([128, sw], BF16, name="p_msk", tag="pmask",
                                   bufs=4, padded_shape=[128, C.SQS])
                    nc.vector.tensor_tensor(
                        out=pm[:], in0=pe[:],
                        in1=maskT[:, j * sw:(j + 1) * sw],
                        op=ALU.mult,
                    )
                    pms.append(pm)
                pm_hist.append((j, pms))
                if len(pm_hist) > PIPE:
                    jj, pp = pm_hist.pop(0)
                    emit_pv(jj, pp)
            for jj, pp in pm_hist:
                emit_pv(jj, pp)
            for i in range(2):
                # Z row 64 -> sbuf; PE-broadcast; approx-reciprocal; scale rows 0..63
                zrow = attn.tile([65, sw], F32, name="zrow", tag="zrow", bufs=1,
                                 padded_shape=[65, C.SQS])
                nc.vector.tensor_copy(zrow[64:65, :], pv[i][64:65, :])
                zb = ps_zb.tile([64, sw], F32, name="zb", tag="s",
                                padded_shape=[128, PS_F])
                for (qs, qw) in slices(sw, 512):
                    nc.tensor.matmul(zb[:, qs:qs + qw], onesf_sb[64:65, 0:64],
                                     zrow[64:65, qs:qs + qw],
                                     start=True, stop=True)
                zr = attn.tile([64, sw], F32, name="zr", tag="zr", bufs=1,
                               padded_shape=[64, C.SQS])
                nc.vector.reciprocal_approx_fast(out=zr[:], in_=zb[:])
                tmp = attn.tile([64, sw], BF16, name="xat_t", tag="xat_t", bufs=1,
                                padded_shape=[64, C.SQS])
                nc.vector.tensor_tensor(out=tmp[:], in0=pv[i][0:64, :],
                                        in1=zr[:], op=ALU.mult)
                # partition hop: rows 0..63 -> xattnT pair rows 64i..64i+64
                nc.sync.dma_start(
                    xattnT_sb[64 * i:64 * (i + 1), hp * C.SQ + sq: hp * C.SQ + sq + sw],
                    tmp[:],
                )

    if C.max_stage <= 3:
        finish()
        return

    # ---------------- stage 4: R = bv @ WoT + bo, broadcast to 128 rows ----
    for (ns, nw) in slices(C.DM, 512):
        psR = ps_zb.tile([1, nw], F32, name="psR", tag="s", padded_shape=[128, PS_F])
        for kt in range(C.KT):
            nc.tensor.matmul(
                psR[:], bvl_sb[:, kt:kt + 1],
                wo_sb[:, kt * C.DM + ns: kt * C.DM + ns + nw],
                start=(kt == 0), stop=(kt == C.KT - 1),
            )
        nc.vector.tensor_tensor(out=Rrow_sb[0:1, ns:ns + nw], in0=psR[:],
                                in1=bo_sb[0:1, ns:ns + nw], op=ALU.add)
        psB = ps_zb.tile([128, nw], F32, name="psB", tag="s", padded_shape=[128, PS_F])
        nc.tensor.matmul(psB[:], onesf_sb[0:1, :], Rrow_sb[0:1, ns:ns + nw],
                         start=True, stop=True)
        nc.vector.tensor_copy(R_sb[:, ns:ns + nw], psB[:])

    # ---------------- stage 5: output projection ----------------
    for m in range(C.SQ // 128):
        for (ns, nw) in slices(C.DM, 512):
            ps = ps_out.tile([128, nw], F32, name="ps_o", tag="s",
                             padded_shape=[128, PS_F])
            for hp in range(C.HP):
                nc.tensor.matmul(
                    ps[:],
                    xattnT_sb[:, hp * C.SQ + m * 128: hp * C.SQ + (m + 1) * 128],
                    wo_sb[:, hp * C.DM + ns: hp * C.DM + ns + nw],
                    start=(hp == 0), stop=(hp == C.HP - 1),
                )
            ot = attn.tile([128, nw], F32, name="out_sb", tag="out_sb", bufs=2,
                           padded_shape=[128, 512])
            nc.vector.tensor_tensor(out=ot[:], in0=ps[:], in1=R_sb[:, ns:ns + nw],
                                    op=ALU.add)
            nc.sync.dma_start(out[m * 128:(m + 1) * 128, ns:ns + nw], ot[:])

    finish()


def build(cfg, reps=1):
    nc = bacc.Bacc("TRN2", target_bir_lowering=False, debug=False)
    C = cfg
    io = {
        "xq": nc.dram_tensor("xq", [C.SQ, C.DM], F32, kind="ExternalInput").ap(),
        "xk": nc.dram_tensor("xk", [C.SK, C.DM], F32, kind="ExternalInput").ap(),
        "xv": nc.dram_tensor("xv", [C.SK, C.DM], F32, kind="ExternalInput").ap(),
        "mask": nc.dram_tensor("mask", [C.SQ, C.SK], I32, kind="ExternalInput").ap(),
        "wqt": nc.dram_tensor("wqt", [C.DM, C.DM], BF16, kind="ExternalInput").ap(),
        "wkt": nc.dram_tensor("wkt", [C.DM, C.DM], BF16, kind="ExternalInput").ap(),
        "wvt": nc.dram_tensor("wvt", [C.DM, C.DM], BF16, kind="ExternalInput").ap(),
        "wot": nc.dram_tensor("wot", [C.DM, C.DM], BF16, kind="ExternalInput").ap(),
        "bql": nc.dram_tensor("bql", [128, C.HP], F32, kind="ExternalInput").ap(),
        "bkl": nc.dram_tensor("bkl", [128, C.HP], F32, kind="ExternalInput").ap(),
        "bvl": nc.dram_tensor("bvl", [128, C.KT], BF16, kind="ExternalInput").ap(),
        "bo_row": nc.dram_tensor("bo_row", [1, C.DM], F32, kind="ExternalInput").ap(),
        "out": nc.dram_tensor("out", [C.SQ, C.DM], F32, kind="ExternalOutput").ap(),
    }
    with tile.TileContext(nc) as tc:
        for _ in range(reps):
            emit_kernel(tc, cfg, io)
    nc.compile()
    return nc


def host_prep(query, key, value, mask, Wq, bq, Wk, bk, Wv, bv, Wo, bo, cfg):
    """Host-side layout prep (weight transpose/cast, per-core slicing)."""
    C = cfg
    bf = ml_dtypes.bfloat16
    wqt = np.ascontiguousarray((Wq.T * 0.125).astype(bf))   # 1/sqrt(dk) folded
    wkt = np.ascontiguousarray(Wk.T.astype(bf))
    wvt = np.ascontiguousarray(Wv.T.astype(bf))
    wot = np.ascontiguousarray(Wo.T.astype(bf))
    bql = np.ascontiguousarray((bq * 0.125).reshape(C.HP, 128).T.astype(np.float32))
    bkl = np.ascontiguousarray(bk.reshape(C.HP, 128).T.astype(np.float32))
    bvl = np.ascontiguousarray(bv.reshape(C.KT, 128).T.astype(bf))
    bo_row = np.ascontiguousarray(bo.reshape(1, C.DM).astype(np.float32))
    shared = dict(wqt=wqt, wkt=wkt, wvt=wvt, wot=wot, bql=bql, bkl=bkl,
                  bvl=bvl, bo_row=bo_row)
    in_maps = []
    B = query.shape[0]
    halves = query.shape[1] // C.SQ
    for c in range(B * halves):
        b, h = divmod(c, halves)
        m = dict(shared)
        m["xq"] = np.ascontiguousarray(query[b, h * C.SQ:(h + 1) * C.SQ, :])
        m["xk"] = np.ascontiguousarray(key[b])
        m["xv"] = np.ascontiguousarray(value[b])
        m["mask"] = np.ascontiguousarray(mask[b, h * C.SQ:(h + 1) * C.SQ, :])
        in_maps.append(m)
    return in_maps


_CACHED = {}


def get_built():
    if "nc" not in _CACHED:
        _CACHED["nc"] = build(Cfg())
    return _CACHED["nc"]


def kernel(query, key, value, mask, Wq, bq, Wk, bk, Wv, bv, Wo, bo):
    cfg = Cfg()
    nc = get_built()
    in_maps = host_prep(query, key, value, mask, Wq, bq, Wk, bk, Wv, bv, Wo, bo, cfg)
    res = run_bass_kernel_spmd(nc, in_maps, core_ids=list(range(N_CORES)))
    B, S, DM = query.shape
    out = np.empty((B, S, DM), np.float32)
    for c in range(N_CORES):
        b, h = divmod(c, 2)
        out[b, h * cfg.SQ:(h + 1) * cfg.SQ, :] = res.results[c]["out"]
    return out
